# revision 20
# baseline (speedup 1.0000x reference)
"""Trainium2 Bass kernel for the GNN edge-update MLP (8 NeuronCores).

Reference semantics:
    h   = x @ W_lin.T + b_lin                       # [N, nin]
    agg = h[src] + h[dst]                           # [E, nin]
    z   = concat([agg, edge_attr], -1)              # [E, 2*nin]
    z   = relu(BN(z @ W1.T + b1; g1, be1))          # [E, nout]  (BN over edges)
    z   = relu(BN(z @ W2.T + b2; g2, be2))          # [E, nout]
Restructuring:
  * b1/b2 cancel inside training-mode BN -> dropped.
  * z @ W1.T = hW[src] + hW[dst] + ea @ W1b.T, with W1 = [W1a | W1b] and
    hW = x @ (W1a W_lin).T + W1a b_lin  (a [N, nout] gather table).
  * Weight prep (WcT = (W1a W_lin)^T, bc = W1a b_lin, W1b^T, W2^T) is done
    on the host in f32.
  * Everything on device is feature-major [128, edges]; host pre-transposes
    edge_attr / x and post-transposes the output.
  * Gathers use GPSIMD dma_gather(transpose=False) -> EDGE-major tiles
    (row i of a group lands in partition i%128, block i//128).  gsrc is
    gathered straight into u1's columns; gdst goes through a small ring and
    is added in place (edge-major) right away, so all 80 gathers stream
    limited only by GPSIMD, overlapping the phase-0 table build and pass A.
    Chunk processing then transposes each 128-edge sub-tile into the
    feature-major PSUM accumulator (identity matmul on top of W1b @ eaT)
    and overwrites u1 with the final feature-major chunk.
  * Gathers are spread over the 4 SWDGE queues (assigned in scheduled order
    so the tile sem lanes stay queue-pure); the 4 Q7 cpu pairs then generate
    descriptors concurrently (~4x).  transpose=True gathers cannot do this
    (they corrupt each other through the shared transpose xbar).
  * int16 gather indices (signed on HW): the table is built in two DRAM
    regions (lo nodes [0,SPLIT), then hi) and the host bucket-sorts each
    core's edges by (src>=SPLIT, dst>=SPLIT) so every gather instruction
    targets one region with small non-negative local indices.  Buckets are
    padded (to the max size over cores) with edges that gather dedicated
    zero rows and have zero edge_attr, so padded u1 columns are exactly 0;
    their (constant) effect on the second BN's statistics is subtracted
    analytically on device.  The (lo,lo) bucket is processed first and its
    gathers only need the lo table, which is built first.
  * Phase 0 writes the node-major tables with per-partition-contiguous
    descriptors: the host permutes xT columns inside each 1024-node chunk
    (col s*128+p holds node 8p+s) so that after the 8 per-block transposes
    SBUF partition p holds nodes 8p..8p+7 contiguously -> each table-write
    descriptor is 1KB instead of 256B.
  * Pass B computes u2 = W2 @ relu(a1*u1+c1) per chunk, takes bn_stats on
    the PSUM f32, and overwrites u1 with u2 (bf16); pass C is then just an
    activation + DMA out (no third matmul pass).  Activation work in passes
    B/C alternates between the Scalar and Vector engines.
  * BN statistics: per-chunk vector bn_stats on PSUM, merged manually,
    AllReduce'd across the 8 cores ([128,2] f32 - tiny).
"""

import sys
from contextlib import ExitStack

import numpy as np

try:
    import concourse  # noqa: F401
except ImportError:  # pragma: no cover
    sys.path.insert(0, "/opt/trn_rl_repo")

import ml_dtypes
from concourse import bass, bacc, mybir, bass_isa
from concourse import tile
from concourse.bass_utils import run_bass_kernel_spmd

BF16 = ml_dtypes.bfloat16

N_CORES = 8
NIN = 128
EPS = 1e-5
P = 128

SPLIT = 32767            # nodes < SPLIT are "lo", >= SPLIT are "hi"
BUCKET_ORDER = (0, 3, 1, 2)   # (lo,lo) first: overlaps the hi-table build
GROUP = 2048             # edges per dma_gather instruction
NQ = 4                   # SWDGE queues
PH0W = 1024              # phase-0 chunk width (nodes)


def table_layout(n_nodes):
    """Two gather tables in DRAM: lo = hW[0:SPLIT) + zero row (padded to
    1024); hi = hW[SPLIT:] + zero row.  xT columns: [lo | hi]."""
    nhi = n_nodes - SPLIT
    lo_rows = ((SPLIT + 1 + 2 * PH0W - 1) // (2 * PH0W)) * (2 * PH0W)
    hi_rows = ((nhi + 1 + 2 * PH0W - 1) // (2 * PH0W)) * (2 * PH0W)
    npad = hi_rows + lo_rows
    return nhi, hi_rows, lo_rows, npad


def edge_layout(caps):
    """Device-side loop structure from bucket capacities.

    Returns (groups, chunks): groups = (off, L, src_hi, dst_hi);
    chunks = (off, S, group_index)."""
    groups = []
    chunks = []
    off = 0
    for b in BUCKET_ORDER:
        src_hi, dst_hi = b >= 2, b % 2 == 1
        rem = caps[b]
        while rem > 0:
            L = min(GROUP, rem)
            gi = len(groups)
            groups.append((off, L, src_hi, dst_hi))
            coff = 0
            while coff < L:
                S = min(512, L - coff)
                chunks.append((off + coff, S, gi))
                coff += S
            off += L
            rem -= L
    return groups, chunks


def _chunks(ec, width):
    out = []
    off = 0
    while off < ec:
        s = min(width, ec - off)
        out.append((off, s))
        off += s
    return out


def build_graph(n_cores, caps, n_nodes, e_total, eps=EPS):
    f32 = mybir.dt.float32
    bf16 = mybir.dt.bfloat16
    i16 = mybir.dt.int16
    FT = mybir.ActivationFunctionType

    nc = bacc.Bacc(
        "TRN2", target_bir_lowering=False, debug=False, num_devices=n_cores,
        num_swdge_queues=NQ,
    )

    nhi, hi_rows, lo_rows, npad = table_layout(n_nodes)
    groups, chunksA = edge_layout(caps)
    ec = sum(caps)
    chunksBC = _chunks(ec, 1024)
    nstatB = sum((S + 511) // 512 for _, S in chunksBC)
    nstat = max(len(chunksA), nstatB)
    n_pad_tot = ec * n_cores - e_total  # padded edges across all cores

    # ---- I/O -------------------------------------------------------------
    eaT = nc.dram_tensor("eaT", [P, ec], bf16, kind="ExternalInput").ap()
    xT = nc.dram_tensor("xT", [P, npad], bf16, kind="ExternalInput").ap()
    sidx = nc.dram_tensor("sidx", [P, ec // 16], i16, kind="ExternalInput").ap()
    didx = nc.dram_tensor("didx", [P, ec // 16], i16, kind="ExternalInput").ap()
    wct = nc.dram_tensor("wct", [P, P], bf16, kind="ExternalInput").ap()
    w1bt = nc.dram_tensor("w1bt", [P, P], bf16, kind="ExternalInput").ap()
    w2t = nc.dram_tensor("w2t", [P, P], bf16, kind="ExternalInput").ap()
    ident = nc.dram_tensor("ident", [P, P], bf16, kind="ExternalInput").ap()
    bc = nc.dram_tensor("bc", [P, 1], f32, kind="ExternalInput").ap()
    g1 = nc.dram_tensor("g1", [P, 1], f32, kind="ExternalInput").ap()
    be1 = nc.dram_tensor("be1", [P, 1], f32, kind="ExternalInput").ap()
    g2 = nc.dram_tensor("g2", [P, 1], f32, kind="ExternalInput").ap()
    be2 = nc.dram_tensor("be2", [P, 1], f32, kind="ExternalInput").ap()
    outT = nc.dram_tensor("outT", [P, ec], bf16, kind="ExternalOutput").ap()

    table_lo = nc.dram_tensor("hw_table_lo", [lo_rows, P], bf16).ap()
    table_hi = nc.dram_tensor("hw_table_hi", [hi_rows, P], bf16).ap()

    grp_all = [list(range(n_cores))]

    with tile.TileContext(nc) as tc, ExitStack() as es:
        consts = es.enter_context(tc.tile_pool(name="consts", bufs=1))
        gidx = es.enter_context(tc.tile_pool(name="gidx", bufs=6))
        dram = es.enter_context(tc.tile_pool(name="dram", bufs=1, space="DRAM"))
        big = es.enter_context(tc.tile_pool(name="big", bufs=1))
        red = es.enter_context(tc.tile_pool(name="red", bufs=1))

        # ---- constants (all weight prep done on host) --------------------
        wct_s = consts.tile([P, P], bf16)
        nc.sync.dma_start(out=wct_s[:], in_=wct)
        w1bt_s = consts.tile([P, P], bf16)
        nc.sync.dma_start(out=w1bt_s[:], in_=w1bt)
        w2t_s = consts.tile([P, P], bf16)
        nc.sync.dma_start(out=w2t_s[:], in_=w2t)
        ident_s = consts.tile([P, P], bf16)
        nc.sync.dma_start(out=ident_s[:], in_=ident)
        bc_s = consts.tile([P, 1], f32)
        nc.sync.dma_start(out=bc_s[:], in_=bc)
        g1_s = consts.tile([P, 1], f32)
        nc.sync.dma_start(out=g1_s[:], in_=g1)
        be1_s = consts.tile([P, 1], f32)
        nc.sync.dma_start(out=be1_s[:], in_=be1)
        g2_s = consts.tile([P, 1], f32)
        nc.sync.dma_start(out=g2_s[:], in_=g2)
        be2_s = consts.tile([P, 1], f32)
        nc.sync.dma_start(out=be2_s[:], in_=be2)
        eps_s = consts.tile([P, 1], f32)
        nc.vector.memset(eps_s[:], eps)

        u1 = big.tile([P, ec], bf16)
        stats = consts.tile([P, nstat, 6], f32)

        # prefetch first few groups' indices
        idx_pre = {}
        for gi, (off, L, _sh, _dh) in enumerate(groups[:3]):
            si = gidx.tile([P, GROUP // 16], i16, tag="si")
            nc.sync.dma_start(out=si[:, :L // 16],
                              in_=sidx[:, off // 16:(off + L) // 16])
            di = gidx.tile([P, GROUP // 16], i16, tag="di")
            nc.sync.dma_start(out=di[:, :L // 16],
                              in_=didx[:, off // 16:(off + L) // 16])
            idx_pre[gi] = (si, di)

        def issue_group(gi, gp):
            off, L, src_hi, dst_hi = groups[gi]
            if gi in idx_pre:
                si, di = idx_pre[gi]
            else:
                si = gidx.tile([P, GROUP // 16], i16, tag="si")
                nc.sync.dma_start(out=si[:, :L // 16],
                                  in_=sidx[:, off // 16:(off + L) // 16])
                di = gidx.tile([P, GROUP // 16], i16, tag="di")
                nc.sync.dma_start(out=di[:, :L // 16],
                                  in_=didx[:, off // 16:(off + L) // 16])
            src_base = table_hi if src_hi else table_lo
            dst_base = table_hi if dst_hi else table_lo
            # gsrc straight into u1's columns (edge-major for now)
            nc.gpsimd.dma_gather(
                out_ap=u1[:, off:off + L].rearrange("p (b f) -> p b f", f=P),
                in_ap=src_base, idxs_ap=si[:, :L // 16],
                num_idxs=L, num_idxs_reg=L, elem_size=P,
                transpose=False, single_packet=False)
            gdst = gp.tile([P, GROUP], bf16, tag="gdst")
            nc.gpsimd.dma_gather(
                out_ap=gdst[:, :L].rearrange("p (b f) -> p b f", f=P),
                in_ap=dst_base, idxs_ap=di[:, :L // 16],
                num_idxs=L, num_idxs_reg=L, elem_size=P,
                transpose=False, single_packet=False)
            nc.vector.tensor_add(u1[:, off:off + L], u1[:, off:off + L],
                                 gdst[:, :L])

        # ---- phase 0: build tables (lo first), gathers streaming ---------
        n_lo_groups = sum(1 for (_, _, sh, dh) in groups if not sh and not dh)
        with tc.tile_pool(name="ph0", bufs=3) as ph0, \
             tc.tile_pool(name="ps0", bufs=2, space="PSUM") as ps0, \
             tc.tile_pool(name="gp", bufs=3) as gp:
            zrow = ph0.tile([P, P], bf16, tag="zrow")
            nc.vector.memset(zrow[:], 0.0)

            def build(tab, xcol0, nch):
                for j2 in range(nch // 2):
                    ts = ph0.tile([P, 2 * PH0W], bf16, tag="ts")
                    for half in range(2):
                        j = 2 * j2 + half
                        xt = ph0.tile([P, PH0W], bf16, tag="xt")
                        nc.sync.dma_start(
                            out=xt[:],
                            in_=xT[:, xcol0 + j * PH0W:xcol0 + (j + 1) * PH0W])
                        hp = ps0.tile([P, PH0W], f32, tag="hp")
                        for h in range(0, PH0W, 512):
                            nc.tensor.matmul(hp[:, h:h + 512], lhsT=wct_s[:],
                                             rhs=xt[:, h:h + 512],
                                             start=True, stop=True)
                        hs = ph0.tile([P, PH0W], bf16, tag="hs")
                        nc.scalar.activation(hs[:], hp[:], func=FT.Identity,
                                             bias=bc_s[:], scale=1.0)
                        tp = ps0.tile([P, PH0W], f32, tag="tp")
                        for s in range(PH0W // P):
                            nc.tensor.matmul(tp[:, s * P:(s + 1) * P],
                                             lhsT=hs[:, s * P:(s + 1) * P],
                                             rhs=ident_s[:], start=True,
                                             stop=True)
                        nc.vector.tensor_copy(
                            ts[:, half * PH0W:(half + 1) * PH0W], tp[:])
                    # partition p holds nodes 8p..8p+7 of both halves -> 1KB
                    # descriptors, one dma_start per 2048 rows
                    nc.sync.dma_start(
                        out=tab[j2 * 2 * PH0W:(j2 + 1) * 2 * PH0W,
                                :].rearrange("(h p s) o -> p h s o", h=2, p=P),
                        in_=ts[:].rearrange("p (h s o) -> p h s o", h=2,
                                            s=PH0W // P),
                    )

            build(table_lo, 0, lo_rows // PH0W)
            nc.sync.dma_start(out=table_lo[SPLIT:SPLIT + 1, :],
                              in_=zrow[0:1, :])
            build(table_hi, lo_rows, hi_rows // PH0W)
            nc.sync.dma_start(out=table_hi[nhi:nhi + 1, :],
                              in_=zrow[0:1, :])
            # (lo,lo) gathers only need the lo table and start streaming as
            # soon as it is written, while the hi table still builds
            for gi in range(len(groups)):
                issue_group(gi, gp)

        def bn_coeffs(g_s, be_s, nchunk, corr=None):
            """Merge bn_stats 6-tuples -> AllReduce -> a, c (scale/bias)."""
            se = red.tile([P, nstat], f32, tag="se")
            nc.vector.tensor_mul(se[:, :nchunk], stats[:, :nchunk, 0],
                                 stats[:, :nchunk, 1])
            so = red.tile([P, nstat], f32, tag="so")
            nc.vector.tensor_mul(so[:, :nchunk], stats[:, :nchunk, 3],
                                 stats[:, :nchunk, 4])
            qe = red.tile([P, nstat], f32, tag="qe")
            nc.vector.tensor_mul(qe[:, :nchunk], se[:, :nchunk],
                                 stats[:, :nchunk, 1])
            nc.vector.tensor_add(qe[:, :nchunk], qe[:, :nchunk],
                                 stats[:, :nchunk, 2])
            qo = red.tile([P, nstat], f32, tag="qo")
            nc.vector.tensor_mul(qo[:, :nchunk], so[:, :nchunk],
                                 stats[:, :nchunk, 4])
            nc.vector.tensor_add(qo[:, :nchunk], qo[:, :nchunk],
                                 stats[:, :nchunk, 5])
            nc.vector.tensor_add(se[:, :nchunk], se[:, :nchunk], so[:, :nchunk])
            nc.vector.tensor_add(qe[:, :nchunk], qe[:, :nchunk], qo[:, :nchunk])
            sq = red.tile([P, 2], f32, tag="sq")
            nc.vector.tensor_reduce(sq[:, 0:1], se[:, :nchunk],
                                    axis=mybir.AxisListType.X,
                                    op=mybir.AluOpType.add)
            nc.vector.tensor_reduce(sq[:, 1:2], qe[:, :nchunk],
                                    axis=mybir.AxisListType.X,
                                    op=mybir.AluOpType.add)
            cc_in = dram.tile([P, 2], f32, tag="cc_in")
            nc.sync.dma_start(out=cc_in[:], in_=sq[:])
            cc_out = dram.tile([P, 2], f32, tag="cc_out")
            nc.gpsimd.collective_compute(
                "AllReduce", mybir.AluOpType.add, replica_groups=grp_all,
                ins=[cc_in[:].opt()], outs=[cc_out[:].opt()])
            sqg = red.tile([P, 2], f32, tag="sqg")
            nc.sync.dma_start(out=sqg[:], in_=cc_out[:])
            if corr is not None:
                # subtract the pad edges' (constant) contribution
                v, vq = corr
                t = red.tile([P, 2], f32, tag="tcorr")
                nc.vector.tensor_scalar_mul(t[:, 0:1], v[:], float(n_pad_tot))
                nc.vector.tensor_scalar_mul(t[:, 1:2], vq[:], float(n_pad_tot))
                nc.vector.tensor_sub(sqg[:], sqg[:], t[:])
            mu = red.tile([P, 1], f32, tag="mu")
            nc.vector.tensor_scalar_mul(mu[:], sqg[:, 0:1], 1.0 / e_total)
            var = red.tile([P, 1], f32, tag="var")
            nc.vector.tensor_scalar_mul(var[:], sqg[:, 1:2], 1.0 / e_total)
            mu2 = red.tile([P, 1], f32, tag="mu2")
            nc.vector.tensor_mul(mu2[:], mu[:], mu[:])
            nc.vector.tensor_sub(var[:], var[:], mu2[:])
            a = red.tile([P, 1], f32, tag="a")
            nc.scalar.activation(a[:], var[:], func=FT.Sqrt, bias=eps_s[:],
                                 scale=1.0)
            nc.vector.reciprocal(a[:], a[:])
            nc.vector.tensor_mul(a[:], a[:], g_s[:])
            c = red.tile([P, 1], f32, tag="c")
            nc.vector.tensor_mul(c[:], mu[:], a[:])
            nc.vector.tensor_sub(c[:], be_s[:], c[:])
            return a, c

        with tc.tile_pool(name="psS", bufs=1, space="PSUM") as psS:
            # ---- pass A: transpose sub-tiles + W1b@eaT -> u1 -------------
            with (
                tc.tile_pool(name="psA", bufs=7, space="PSUM") as psA,
                tc.tile_pool(name="ea", bufs=8) as eap,
            ):
                for k, (off, S, gi) in enumerate(chunksA):
                    ea_t = eap.tile([P, 512], bf16, tag="ea")
                    nc.sync.dma_start(out=ea_t[:, :S],
                                      in_=eaT[:, off:off + S])
                    up = psA.tile([P, 512], f32, tag="up")
                    nc.tensor.matmul(up[:, :S], lhsT=w1bt_s[:],
                                     rhs=ea_t[:, :S], start=True, stop=False)
                    for s in range(S // P):
                        nc.tensor.matmul(
                            up[:, s * P:(s + 1) * P],
                            lhsT=u1[:, off + s * P:off + (s + 1) * P],
                            rhs=ident_s[:], start=False, stop=True)
                    nc.vector.bn_stats(stats[:, k, :], up[:, :S])
                    nc.scalar.activation(u1[:, off:off + S], up[:, :S],
                                         func=FT.Identity)

                a1, c1 = bn_coeffs(g1_s, be1_s, len(chunksA))

            # pad columns have u1 == 0 -> u2_pad = W2 @ relu(c1), constant
            rc = red.tile([P, 1], f32, tag="rc")
            nc.scalar.activation(rc[:], c1[:], func=FT.Relu)
            rcb = red.tile([P, 1], bf16, tag="rcb")
            nc.vector.tensor_copy(rcb[:], rc[:])
            vp = psS.tile([P, 1], f32, tag="vp")
            nc.tensor.matmul(vp[:], lhsT=w2t_s[:], rhs=rcb[:],
                             start=True, stop=True)
            v2 = red.tile([P, 1], f32, tag="v2")
            nc.vector.tensor_copy(v2[:], vp[:])
            v2q = red.tile([P, 1], f32, tag="v2q")
            nc.vector.tensor_mul(v2q[:], v2[:], v2[:])

            # ---- pass B: z1 = relu(a1*u1+c1); u2 = W2@z1 overwrites u1 ---
            with tc.tile_pool(name="psB", bufs=3, space="PSUM") as psB:
                kb = 0
                for ci, (off, S) in enumerate(chunksBC):
                    if ci % 6 != 5:
                        nc.scalar.activation(u1[:, off:off + S],
                                             u1[:, off:off + S],
                                             func=FT.Relu, scale=a1[:],
                                             bias=c1[:])
                    else:
                        nc.vector.tensor_scalar(
                            u1[:, off:off + S], u1[:, off:off + S],
                            scalar1=a1[:], scalar2=c1[:],
                            op0=mybir.AluOpType.mult,
                            op1=mybir.AluOpType.add)
                        nc.vector.tensor_scalar_max(u1[:, off:off + S],
                                                    u1[:, off:off + S], 0.0)
                    up = psB.tile([P, 1024], f32, tag="upb")
                    for h in range(0, S, 512):
                        hs_ = min(512, S - h)
                        nc.tensor.matmul(up[:, h:h + hs_], lhsT=w2t_s[:],
                                         rhs=u1[:, off + h:off + h + hs_],
                                         start=True, stop=True)
                        nc.vector.bn_stats(stats[:, kb, :],
                                           up[:, h:h + hs_])
                        kb += 1
                    # copy-back alternates engines to balance load
                    if ci % 3 < 2:
                        nc.scalar.activation(u1[:, off:off + S], up[:, :S],
                                             func=FT.Identity)
                    else:
                        nc.vector.tensor_copy(u1[:, off:off + S], up[:, :S])

                a2, c2 = bn_coeffs(g2_s, be2_s, kb, corr=(v2, v2q))

        # ---- pass C: out = relu(a2*u2+c2) --------------------------------
        with tc.tile_pool(name="op", bufs=6) as op:
            for ci, (off, S) in enumerate(chunksBC):
                ot = op.tile([P, 1024], bf16, tag="ot")
                if ci % 3 != 2:
                    nc.scalar.activation(ot[:, :S], u1[:, off:off + S],
                                         func=FT.Relu, scale=a2[:],
                                         bias=c2[:])
                else:
                    nc.vector.tensor_scalar(
                        ot[:, :S], u1[:, off:off + S], scalar1=a2[:],
                        scalar2=c2[:], op0=mybir.AluOpType.mult,
                        op1=mybir.AluOpType.add)
                    nc.vector.tensor_scalar_max(ot[:, :S], ot[:, :S], 0.0)
                nc.sync.dma_start(out=outT[:, off:off + S], in_=ot[:, :S])

    # Spread SWDGE gathers over the 4 queues (Q7 cpu pairs) so descriptor
    # generation runs 4-wide.  Assign queue = k % NQ in SCHEDULED order: the
    # tile sem pass assigns completion-sem lanes round-robin (k % 8) over
    # Pool-engine DMA instructions in the same order, so every sem lane sees
    # exactly one queue and completions stay in-order per lane.
    k = 0
    for insts in tc.ordered_instructions_by_block.values():
        for inst in insts:
            if (isinstance(inst, bass_isa.AnyDMAInstruction)
                    and inst.engine == mybir.EngineType.Pool):
                assert isinstance(inst, mybir.InstDMAGatherAnt), type(inst)
                inst.queue_num = k % NQ
                k += 1

    nc.compile()
    return nc


def _wrap16(a):
    """linear [L] -> [16, L/16] wrapped, tiled to [128, L/16]."""
    w = np.ascontiguousarray(a.reshape(-1, 16).T)
    return np.tile(w, (8, 1))


def _perm_cols(xpad):
    """[rows, 128] node-major -> xT columns with the phase-0 permutation:
    within each 1024-node chunk, column s*128+p holds node 8p+s."""
    rows = xpad.shape[0]
    a = xpad.reshape(rows // PH0W, P, PH0W // P, P)   # [j, p, s, f]
    a = a.transpose(0, 2, 1, 3).reshape(rows, P)      # [j, s, p, f]
    return a.T                                        # [128, rows]


def host_prep(x, edge_index, edge_attr, n_cores):
    """Shard edges, bucket-sort, pad; returns per-core arrays + caps."""
    n = x.shape[0]
    e = edge_attr.shape[0]
    ec0 = e // n_cores
    nhi, hi_rows, lo_rows, npad = table_layout(n)

    src_all = edge_index[0].astype(np.int64)
    dst_all = edge_index[1].astype(np.int64)

    per_core = []
    counts = np.zeros((n_cores, 4), np.int64)
    for c in range(n_cores):
        sl = slice(c * ec0, (c + 1) * ec0)
        s, d = src_all[sl], dst_all[sl]
        key = (s >= SPLIT) * 2 + (d >= SPLIT)
        order = np.argsort(key, kind="stable")
        counts[c] = np.bincount(key, minlength=4)
        per_core.append((s, d, key, order))

    caps = tuple(int(max(128, ((counts[:, b].max() + 127) // 128) * 128))
                 for b in range(4))
    ec = sum(caps)
    offs = {}
    _acc = 0
    for b in BUCKET_ORDER:
        offs[b] = _acc
        _acc += caps[b]

    zero_lo = SPLIT          # local zero-row idx in the lo region
    zero_hi = nhi            # local zero-row idx in the hi region

    cores = []
    for c in range(n_cores):
        s, d, key, order = per_core[c]
        cnt = counts[c]
        pos_sorted = np.empty(ec0, np.int64)
        start = 0
        sidx_p = np.empty(ec, np.int64)
        didx_p = np.empty(ec, np.int64)
        ea_cols = np.full(ec, -1, np.int64)  # source edge for each padded col
        for b in range(4):
            idx_b = order[start:start + cnt[b]]
            pos = offs[b] + np.arange(cnt[b])
            pos_sorted[start:start + cnt[b]] = pos
            sb = s[idx_b]
            db = d[idx_b]
            src_hi, dst_hi = b >= 2, b % 2 == 1
            sl_loc = sb - SPLIT if src_hi else sb
            dl_loc = db - SPLIT if dst_hi else db
            sidx_p[pos] = sl_loc
            didx_p[pos] = dl_loc
            ea_cols[pos] = idx_b
            padr = np.arange(offs[b] + cnt[b], offs[b] + caps[b])
            sidx_p[padr] = zero_hi if src_hi else zero_lo
            didx_p[padr] = zero_hi if dst_hi else zero_lo
            start += cnt[b]
        inv = np.empty(ec0, np.int64)
        inv[order] = pos_sorted  # padded position of original local edge
        cores.append((sidx_p.astype(np.int16), didx_p.astype(np.int16),
                      ea_cols, inv))
    return caps, ec, cores, npad


def make_in_maps(x, edge_index, edge_attr, W_lin, b_lin, W1, g1, be1, W2,
                 g2, be2, n_cores):
    n = x.shape[0]
    nhi, hi_rows, lo_rows, npad = table_layout(n)
    caps, ec, cores, _ = host_prep(x, edge_index, edge_attr, n_cores)

    # xT columns: [0, lo_rows) lo nodes, [lo_rows, npad) hi nodes, both with
    # the per-1024-chunk phase-0 permutation.
    xbf = x.astype(BF16)
    xlo = np.zeros((lo_rows, P), dtype=BF16)
    xlo[:SPLIT] = xbf[:SPLIT]
    xhi = np.zeros((hi_rows, P), dtype=BF16)
    xhi[:nhi] = xbf[SPLIT:n]
    xT = np.concatenate([_perm_cols(xlo), _perm_cols(xhi)], axis=1)
    xT = np.ascontiguousarray(xT)

    # host-side weight prep (f32)
    W_lin = np.asarray(W_lin, np.float32)
    b_lin = np.asarray(b_lin, np.float32)
    W1 = np.asarray(W1, np.float32)
    W2 = np.asarray(W2, np.float32)
    W1a = W1[:, :P]
    W1b = W1[:, P:]
    Wc = W1a @ W_lin                     # [nout, nin]
    bc_h = (W1a @ b_lin).astype(np.float32).reshape(P, 1)
    wct_h = np.ascontiguousarray(Wc.T).astype(BF16)
    w1bt_h = np.ascontiguousarray(W1b.T).astype(BF16)
    w2t_h = np.ascontiguousarray(W2.T).astype(BF16)
    ident_h = np.eye(P, dtype=BF16)

    f32c = np.ascontiguousarray
    g1_h = f32c(np.asarray(g1, np.float32).reshape(P, 1))
    be1_h = f32c(np.asarray(be1, np.float32).reshape(P, 1))
    g2_h = f32c(np.asarray(g2, np.float32).reshape(P, 1))
    be2_h = f32c(np.asarray(be2, np.float32).reshape(P, 1))

    groups, _ = edge_layout(caps)
    eabf = edge_attr.astype(BF16)

    in_maps = []
    invs = []
    for c in range(n_cores):
        sidx_p, didx_p, ea_cols, inv = cores[c]
        ec0 = inv.shape[0]
        eaT = np.zeros((P, ec), dtype=BF16)
        real = ea_cols >= 0
        eaT[:, real] = eabf[c * ec0 + ea_cols[real]].T
        sw = np.zeros((P, ec // 16), np.int16)
        dw = np.zeros((P, ec // 16), np.int16)
        for off, L, _, _ in groups:
            sw[:, off // 16:(off + L) // 16] = _wrap16(sidx_p[off:off + L])
            dw[:, off // 16:(off + L) // 16] = _wrap16(didx_p[off:off + L])
        in_maps.append({
            "eaT": eaT, "xT": xT, "sidx": sw, "didx": dw,
            "wct": wct_h, "w1bt": w1bt_h, "w2t": w2t_h, "ident": ident_h,
            "bc": bc_h, "g1": g1_h, "be1": be1_h, "g2": g2_h, "be2": be2_h,
        })
        invs.append(inv)
    return caps, ec, in_maps, invs


_GRAPH_CACHE = {}


def get_graph(n_cores, caps, n_nodes, e_total):
    key = (n_cores, caps, n_nodes, e_total)
    if key not in _GRAPH_CACHE:
        _GRAPH_CACHE[key] = build_graph(n_cores, caps, n_nodes, e_total)
    return _GRAPH_CACHE[key]


def kernel(x, edge_index, edge_attr, W_lin, b_lin, W1, b1, g1, be1, W2, b2,
           g2, be2):
    """Full-input entry point: shard, run on 8 NeuronCores, gather."""
    x = np.asarray(x)
    edge_index = np.asarray(edge_index)
    edge_attr = np.asarray(edge_attr)
    e = edge_attr.shape[0]
    n = x.shape[0]
    ec0 = e // N_CORES

    caps, ec, in_maps, invs = make_in_maps(
        x, edge_index, edge_attr, np.asarray(W_lin), np.asarray(b_lin),
        np.asarray(W1), np.asarray(g1), np.asarray(be1), np.asarray(W2),
        np.asarray(g2), np.asarray(be2), N_CORES)
    nc = get_graph(N_CORES, caps, n, e)
    res = run_bass_kernel_spmd(nc, in_maps, core_ids=list(range(N_CORES)))
    out = np.empty((e, NIN), dtype=np.float32)
    for c in range(N_CORES):
        oT = np.asarray(res.results[c]["outT"], dtype=np.float32)
        out[c * ec0:(c + 1) * ec0] = oT.T[invs[c]]
    return out


# revision 23
# speedup vs baseline: 1.0334x; 1.0334x over previous
"""Trainium2 Bass kernel for the GNN edge-update MLP (8 NeuronCores).

Reference semantics:
    h   = x @ W_lin.T + b_lin                       # [N, nin]
    agg = h[src] + h[dst]                           # [E, nin]
    z   = concat([agg, edge_attr], -1)              # [E, 2*nin]
    z   = relu(BN(z @ W1.T + b1; g1, be1))          # [E, nout]  (BN over edges)
    z   = relu(BN(z @ W2.T + b2; g2, be2))          # [E, nout]
Restructuring:
  * b1/b2 cancel inside training-mode BN -> dropped.
  * z @ W1.T = hW[src] + hW[dst] + ea @ W1b.T, with W1 = [W1a | W1b] and
    hW = x @ (W1a W_lin).T + W1a b_lin  (a [N, nout] gather table).
  * Weight prep (WcT = (W1a W_lin)^T, bc = W1a b_lin, W1b^T, W2^T) is done
    on the host in f32.
  * Everything on device is feature-major [128, edges]; host pre-transposes
    edge_attr / x and post-transposes the output.
  * Gathers use GPSIMD dma_gather(transpose=False) -> EDGE-major tiles
    (row i of a group lands in partition i%128, block i//128).  gsrc is
    gathered straight into u1's columns; gdst goes through a small ring and
    is added in place (edge-major) right away, so all 80 gathers stream
    limited only by GPSIMD, overlapping the phase-0 table build and pass A.
    Chunk processing then transposes each 128-edge sub-tile into the
    feature-major PSUM accumulator (identity matmul on top of W1b @ eaT)
    and overwrites u1 with the final feature-major chunk.
  * Gathers are spread over the 4 SWDGE queues (assigned in scheduled order
    so the tile sem lanes stay queue-pure); the 4 Q7 cpu pairs then generate
    descriptors concurrently (~4x).  transpose=True gathers cannot do this
    (they corrupt each other through the shared transpose xbar).
  * int16 gather indices (signed on HW): the table is built in two DRAM
    regions (lo nodes [0,SPLIT), then hi) and the host bucket-sorts each
    core's edges by (src>=SPLIT, dst>=SPLIT) so every gather instruction
    targets one region with small non-negative local indices.  Buckets are
    padded (to the max size over cores) with edges that gather dedicated
    zero rows and have zero edge_attr, so padded u1 columns are exactly 0;
    their (constant) effect on the second BN's statistics is subtracted
    analytically on device.  The (lo,lo) bucket is processed first and its
    gathers only need the lo table, which is built first.
  * Phase 0 writes the node-major tables with per-partition-contiguous
    descriptors: the host permutes xT columns inside each 1024-node chunk
    (col s*128+p holds node 8p+s) so that after the 8 per-block transposes
    SBUF partition p holds nodes 8p..8p+7 contiguously -> each table-write
    descriptor is 1KB instead of 256B.
  * Pass B computes u2 = W2 @ relu(a1*u1+c1) per chunk, takes bn_stats on
    the PSUM f32, and overwrites u1 with u2 (bf16); pass C is then just an
    activation + DMA out (no third matmul pass).  Activation work in passes
    B/C alternates between the Scalar and Vector engines.
  * BN statistics: per-chunk vector bn_stats on PSUM, merged manually,
    AllReduce'd across the 8 cores ([128,2] f32 - tiny).
"""

import sys
from contextlib import ExitStack

import numpy as np

try:
    import concourse  # noqa: F401
except ImportError:  # pragma: no cover
    sys.path.insert(0, "/opt/trn_rl_repo")

import ml_dtypes
from concourse import bass, bacc, mybir, bass_isa
from concourse import tile
from concourse.bass_utils import run_bass_kernel_spmd

BF16 = ml_dtypes.bfloat16

N_CORES = 8
NIN = 128
EPS = 1e-5
P = 128

SPLIT = 32767            # nodes < SPLIT are "lo", >= SPLIT are "hi"
BUCKET_ORDER = (0, 3, 1, 2)   # (lo,lo) first: overlaps the hi-table build
GROUP = 2048             # edges per dma_gather instruction
NQ = 4                   # SWDGE queues
PH0W = 1024              # phase-0 chunk width (nodes)


def table_layout(n_nodes):
    """Two gather tables in DRAM: lo = hW[0:SPLIT) + zero row (padded to
    1024); hi = hW[SPLIT:] + zero row.  xT columns: [lo | hi]."""
    nhi = n_nodes - SPLIT
    lo_rows = ((SPLIT + 1 + 2 * PH0W - 1) // (2 * PH0W)) * (2 * PH0W)
    hi_rows = ((nhi + 1 + 2 * PH0W - 1) // (2 * PH0W)) * (2 * PH0W)
    npad = hi_rows + lo_rows
    return nhi, hi_rows, lo_rows, npad


def edge_layout(caps):
    """Device-side loop structure from bucket capacities.

    Returns (groups, chunks): groups = (off, L, src_hi, dst_hi);
    chunks = (off, S, group_index)."""
    groups = []
    chunks = []
    off = 0
    for b in BUCKET_ORDER:
        src_hi, dst_hi = b >= 2, b % 2 == 1
        rem = caps[b]
        while rem > 0:
            L = min(GROUP, rem)
            gi = len(groups)
            groups.append((off, L, src_hi, dst_hi))
            coff = 0
            while coff < L:
                S = min(512, L - coff)
                chunks.append((off + coff, S, gi))
                coff += S
            off += L
            rem -= L
    return groups, chunks


def _chunks(ec, width):
    out = []
    off = 0
    while off < ec:
        s = min(width, ec - off)
        out.append((off, s))
        off += s
    return out


def build_graph(n_cores, caps, n_nodes, e_total, eps=EPS):
    f32 = mybir.dt.float32
    bf16 = mybir.dt.bfloat16
    i16 = mybir.dt.int16
    FT = mybir.ActivationFunctionType

    nc = bacc.Bacc(
        "TRN2", target_bir_lowering=False, debug=False, num_devices=n_cores,
        num_swdge_queues=NQ,
    )

    nhi, hi_rows, lo_rows, npad = table_layout(n_nodes)
    groups, chunksA = edge_layout(caps)
    ec = sum(caps)
    chunksBC = _chunks(ec, 1024)
    nstatB = sum((S + 511) // 512 for _, S in chunksBC)
    nstat = max(len(chunksA), nstatB)
    n_pad_tot = ec * n_cores - e_total  # padded edges across all cores

    # ---- I/O -------------------------------------------------------------
    eaT = nc.dram_tensor("eaT", [P, ec], bf16, kind="ExternalInput").ap()
    xT = nc.dram_tensor("xT", [P, npad], bf16, kind="ExternalInput").ap()
    sidx = nc.dram_tensor("sidx", [P, ec // 16], i16, kind="ExternalInput").ap()
    didx = nc.dram_tensor("didx", [P, ec // 16], i16, kind="ExternalInput").ap()
    wct = nc.dram_tensor("wct", [P, P], bf16, kind="ExternalInput").ap()
    w1bt = nc.dram_tensor("w1bt", [P, P], bf16, kind="ExternalInput").ap()
    w2t = nc.dram_tensor("w2t", [P, P], bf16, kind="ExternalInput").ap()
    ident = nc.dram_tensor("ident", [P, P], bf16, kind="ExternalInput").ap()
    bc = nc.dram_tensor("bc", [P, 1], f32, kind="ExternalInput").ap()
    g1 = nc.dram_tensor("g1", [P, 1], f32, kind="ExternalInput").ap()
    be1 = nc.dram_tensor("be1", [P, 1], f32, kind="ExternalInput").ap()
    g2 = nc.dram_tensor("g2", [P, 1], f32, kind="ExternalInput").ap()
    be2 = nc.dram_tensor("be2", [P, 1], f32, kind="ExternalInput").ap()
    outT = nc.dram_tensor("outT", [P, ec], bf16, kind="ExternalOutput").ap()

    table_lo = nc.dram_tensor("hw_table_lo", [lo_rows, P], bf16).ap()
    table_hi = nc.dram_tensor("hw_table_hi", [hi_rows, P], bf16).ap()

    grp_all = [list(range(n_cores))]

    with tile.TileContext(nc) as tc, ExitStack() as es:
        consts = es.enter_context(tc.tile_pool(name="consts", bufs=1))
        gidx = es.enter_context(tc.tile_pool(name="gidx", bufs=6))
        dram = es.enter_context(tc.tile_pool(name="dram", bufs=1, space="DRAM"))
        big = es.enter_context(tc.tile_pool(name="big", bufs=1))
        red = es.enter_context(tc.tile_pool(name="red", bufs=1))

        # ---- constants (all weight prep done on host) --------------------
        wct_s = consts.tile([P, P], bf16)
        nc.sync.dma_start(out=wct_s[:], in_=wct)
        w1bt_s = consts.tile([P, P], bf16)
        nc.sync.dma_start(out=w1bt_s[:], in_=w1bt)
        w2t_s = consts.tile([P, P], bf16)
        nc.sync.dma_start(out=w2t_s[:], in_=w2t)
        ident_s = consts.tile([P, P], bf16)
        nc.sync.dma_start(out=ident_s[:], in_=ident)
        bc_s = consts.tile([P, 1], f32)
        nc.sync.dma_start(out=bc_s[:], in_=bc)
        g1_s = consts.tile([P, 1], f32)
        nc.sync.dma_start(out=g1_s[:], in_=g1)
        be1_s = consts.tile([P, 1], f32)
        nc.sync.dma_start(out=be1_s[:], in_=be1)
        g2_s = consts.tile([P, 1], f32)
        nc.sync.dma_start(out=g2_s[:], in_=g2)
        be2_s = consts.tile([P, 1], f32)
        nc.sync.dma_start(out=be2_s[:], in_=be2)
        eps_s = consts.tile([P, 1], f32)
        nc.vector.memset(eps_s[:], eps)

        u1 = big.tile([P, ec], bf16)
        stats = consts.tile([P, nstat, 6], f32)

        # prefetch first few groups' indices
        idx_pre = {}
        for gi, (off, L, _sh, _dh) in enumerate(groups[:3]):
            si = gidx.tile([P, GROUP // 16], i16, tag="si")
            nc.sync.dma_start(out=si[:, :L // 16],
                              in_=sidx[:, off // 16:(off + L) // 16])
            di = gidx.tile([P, GROUP // 16], i16, tag="di")
            nc.sync.dma_start(out=di[:, :L // 16],
                              in_=didx[:, off // 16:(off + L) // 16])
            idx_pre[gi] = (si, di)

        def issue_group(gi, gp):
            off, L, src_hi, dst_hi = groups[gi]
            if gi in idx_pre:
                si, di = idx_pre[gi]
            else:
                si = gidx.tile([P, GROUP // 16], i16, tag="si")
                nc.sync.dma_start(out=si[:, :L // 16],
                                  in_=sidx[:, off // 16:(off + L) // 16])
                di = gidx.tile([P, GROUP // 16], i16, tag="di")
                nc.sync.dma_start(out=di[:, :L // 16],
                                  in_=didx[:, off // 16:(off + L) // 16])
            src_base = table_hi if src_hi else table_lo
            dst_base = table_hi if dst_hi else table_lo
            # gsrc straight into u1's columns (edge-major for now)
            nc.gpsimd.dma_gather(
                out_ap=u1[:, off:off + L].rearrange("p (b f) -> p b f", f=P),
                in_ap=src_base, idxs_ap=si[:, :L // 16],
                num_idxs=L, num_idxs_reg=L, elem_size=P,
                transpose=False, single_packet=False)
            gdst = gp.tile([P, GROUP], bf16, tag="gdst")
            nc.gpsimd.dma_gather(
                out_ap=gdst[:, :L].rearrange("p (b f) -> p b f", f=P),
                in_ap=dst_base, idxs_ap=di[:, :L // 16],
                num_idxs=L, num_idxs_reg=L, elem_size=P,
                transpose=False, single_packet=False)
            nc.vector.tensor_add(u1[:, off:off + L], u1[:, off:off + L],
                                 gdst[:, :L])

        # ---- phase 0: build tables (lo first), gathers streaming ---------
        n_lo_groups = sum(1 for (_, _, sh, dh) in groups if not sh and not dh)
        with tc.tile_pool(name="ph0", bufs=3) as ph0, \
             tc.tile_pool(name="ps0", bufs=2, space="PSUM") as ps0, \
             tc.tile_pool(name="gp", bufs=3) as gp:
            zrow = ph0.tile([P, P], bf16, tag="zrow")
            nc.vector.memset(zrow[:], 0.0)

            def build(tab, xcol0, nch):
                for j2 in range(nch // 2):
                    ts = ph0.tile([P, 2 * PH0W], bf16, tag="ts")
                    for half in range(2):
                        j = 2 * j2 + half
                        xt = ph0.tile([P, PH0W], bf16, tag="xt")
                        nc.sync.dma_start(
                            out=xt[:],
                            in_=xT[:, xcol0 + j * PH0W:xcol0 + (j + 1) * PH0W])
                        hp = ps0.tile([P, PH0W], f32, tag="hp")
                        for h in range(0, PH0W, 512):
                            nc.tensor.matmul(hp[:, h:h + 512], lhsT=wct_s[:],
                                             rhs=xt[:, h:h + 512],
                                             start=True, stop=True)
                        hs = ph0.tile([P, PH0W], bf16, tag="hs")
                        nc.scalar.activation(hs[:], hp[:], func=FT.Identity,
                                             bias=bc_s[:], scale=1.0)
                        tp = ps0.tile([P, PH0W], f32, tag="tp")
                        for s in range(PH0W // P):
                            nc.tensor.matmul(tp[:, s * P:(s + 1) * P],
                                             lhsT=hs[:, s * P:(s + 1) * P],
                                             rhs=ident_s[:], start=True,
                                             stop=True)
                        nc.vector.tensor_copy(
                            ts[:, half * PH0W:(half + 1) * PH0W], tp[:])
                    # partition p holds nodes 8p..8p+7 of both halves -> 1KB
                    # descriptors, one dma_start per 2048 rows
                    nc.sync.dma_start(
                        out=tab[j2 * 2 * PH0W:(j2 + 1) * 2 * PH0W,
                                :].rearrange("(h p s) o -> p h s o", h=2, p=P),
                        in_=ts[:].rearrange("p (h s o) -> p h s o", h=2,
                                            s=PH0W // P),
                    )

            build(table_lo, 0, lo_rows // PH0W)
            nc.sync.dma_start(out=table_lo[SPLIT:SPLIT + 1, :],
                              in_=zrow[0:1, :])
            # (lo,lo) gathers stream while the hi table builds; issue the hi
            # build after a few groups so table_hi lands before the lo
            # buckets run dry
            n_pre = min(8, n_lo_groups)
            for gi in range(n_pre):
                issue_group(gi, gp)
            build(table_hi, lo_rows, hi_rows // PH0W)
            nc.sync.dma_start(out=table_hi[nhi:nhi + 1, :],
                              in_=zrow[0:1, :])
            for gi in range(n_pre, len(groups)):
                issue_group(gi, gp)

        def bn_coeffs(g_s, be_s, nchunk, corr=None):
            """Merge bn_stats 6-tuples -> AllReduce -> a, c (scale/bias)."""
            se = red.tile([P, nstat], f32, tag="se")
            nc.vector.tensor_mul(se[:, :nchunk], stats[:, :nchunk, 0],
                                 stats[:, :nchunk, 1])
            so = red.tile([P, nstat], f32, tag="so")
            nc.vector.tensor_mul(so[:, :nchunk], stats[:, :nchunk, 3],
                                 stats[:, :nchunk, 4])
            qe = red.tile([P, nstat], f32, tag="qe")
            nc.vector.tensor_mul(qe[:, :nchunk], se[:, :nchunk],
                                 stats[:, :nchunk, 1])
            nc.vector.tensor_add(qe[:, :nchunk], qe[:, :nchunk],
                                 stats[:, :nchunk, 2])
            qo = red.tile([P, nstat], f32, tag="qo")
            nc.vector.tensor_mul(qo[:, :nchunk], so[:, :nchunk],
                                 stats[:, :nchunk, 4])
            nc.vector.tensor_add(qo[:, :nchunk], qo[:, :nchunk],
                                 stats[:, :nchunk, 5])
            nc.vector.tensor_add(se[:, :nchunk], se[:, :nchunk], so[:, :nchunk])
            nc.vector.tensor_add(qe[:, :nchunk], qe[:, :nchunk], qo[:, :nchunk])
            sq = red.tile([P, 2], f32, tag="sq")
            nc.vector.tensor_reduce(sq[:, 0:1], se[:, :nchunk],
                                    axis=mybir.AxisListType.X,
                                    op=mybir.AluOpType.add)
            nc.vector.tensor_reduce(sq[:, 1:2], qe[:, :nchunk],
                                    axis=mybir.AxisListType.X,
                                    op=mybir.AluOpType.add)
            cc_in = dram.tile([P, 2], f32, tag="cc_in")
            nc.sync.dma_start(out=cc_in[:], in_=sq[:])
            cc_out = dram.tile([P, 2], f32, tag="cc_out")
            nc.gpsimd.collective_compute(
                "AllReduce", mybir.AluOpType.add, replica_groups=grp_all,
                ins=[cc_in[:].opt()], outs=[cc_out[:].opt()])
            sqg = red.tile([P, 2], f32, tag="sqg")
            nc.sync.dma_start(out=sqg[:], in_=cc_out[:])
            if corr is not None:
                # subtract the pad edges' (constant) contribution
                v, vq = corr
                t = red.tile([P, 2], f32, tag="tcorr")
                nc.vector.tensor_scalar_mul(t[:, 0:1], v[:], float(n_pad_tot))
                nc.vector.tensor_scalar_mul(t[:, 1:2], vq[:], float(n_pad_tot))
                nc.vector.tensor_sub(sqg[:], sqg[:], t[:])
            mu = red.tile([P, 1], f32, tag="mu")
            nc.vector.tensor_scalar_mul(mu[:], sqg[:, 0:1], 1.0 / e_total)
            var = red.tile([P, 1], f32, tag="var")
            nc.vector.tensor_scalar_mul(var[:], sqg[:, 1:2], 1.0 / e_total)
            mu2 = red.tile([P, 1], f32, tag="mu2")
            nc.vector.tensor_mul(mu2[:], mu[:], mu[:])
            nc.vector.tensor_sub(var[:], var[:], mu2[:])
            a = red.tile([P, 1], f32, tag="a")
            nc.scalar.activation(a[:], var[:], func=FT.Sqrt, bias=eps_s[:],
                                 scale=1.0)
            nc.vector.reciprocal(a[:], a[:])
            nc.vector.tensor_mul(a[:], a[:], g_s[:])
            c = red.tile([P, 1], f32, tag="c")
            nc.vector.tensor_mul(c[:], mu[:], a[:])
            nc.vector.tensor_sub(c[:], be_s[:], c[:])
            return a, c

        with tc.tile_pool(name="psS", bufs=1, space="PSUM") as psS:
            # ---- pass A: transpose sub-tiles + W1b@eaT -> u1 -------------
            with (
                tc.tile_pool(name="psA", bufs=7, space="PSUM") as psA,
                tc.tile_pool(name="ea", bufs=8) as eap,
            ):
                for k, (off, S, gi) in enumerate(chunksA):
                    ea_t = eap.tile([P, 512], bf16, tag="ea")
                    nc.sync.dma_start(out=ea_t[:, :S],
                                      in_=eaT[:, off:off + S])
                    up = psA.tile([P, 512], f32, tag="up")
                    nc.tensor.matmul(up[:, :S], lhsT=w1bt_s[:],
                                     rhs=ea_t[:, :S], start=True, stop=False)
                    for s in range(S // P):
                        nc.tensor.matmul(
                            up[:, s * P:(s + 1) * P],
                            lhsT=u1[:, off + s * P:off + (s + 1) * P],
                            rhs=ident_s[:], start=False, stop=True)
                    nc.vector.bn_stats(stats[:, k, :], up[:, :S])
                    nc.scalar.activation(u1[:, off:off + S], up[:, :S],
                                         func=FT.Identity)

                a1, c1 = bn_coeffs(g1_s, be1_s, len(chunksA))

            # pad columns have u1 == 0 -> u2_pad = W2 @ relu(c1), constant
            rc = red.tile([P, 1], f32, tag="rc")
            nc.scalar.activation(rc[:], c1[:], func=FT.Relu)
            rcb = red.tile([P, 1], bf16, tag="rcb")
            nc.vector.tensor_copy(rcb[:], rc[:])
            vp = psS.tile([P, 1], f32, tag="vp")
            nc.tensor.matmul(vp[:], lhsT=w2t_s[:], rhs=rcb[:],
                             start=True, stop=True)
            v2 = red.tile([P, 1], f32, tag="v2")
            nc.vector.tensor_copy(v2[:], vp[:])
            v2q = red.tile([P, 1], f32, tag="v2q")
            nc.vector.tensor_mul(v2q[:], v2[:], v2[:])

            # ---- pass B: z1 = relu(a1*u1+c1); u2 = W2@z1 overwrites u1 ---
            with tc.tile_pool(name="psB", bufs=3, space="PSUM") as psB:
                kb = 0
                for ci, (off, S) in enumerate(chunksBC):
                    if ci % 6 != 5:
                        nc.scalar.activation(u1[:, off:off + S],
                                             u1[:, off:off + S],
                                             func=FT.Relu, scale=a1[:],
                                             bias=c1[:])
                    else:
                        nc.vector.tensor_scalar(
                            u1[:, off:off + S], u1[:, off:off + S],
                            scalar1=a1[:], scalar2=c1[:],
                            op0=mybir.AluOpType.mult,
                            op1=mybir.AluOpType.add)
                        nc.vector.tensor_scalar_max(u1[:, off:off + S],
                                                    u1[:, off:off + S], 0.0)
                    up = psB.tile([P, 1024], f32, tag="upb")
                    for h in range(0, S, 512):
                        hs_ = min(512, S - h)
                        nc.tensor.matmul(up[:, h:h + hs_], lhsT=w2t_s[:],
                                         rhs=u1[:, off + h:off + h + hs_],
                                         start=True, stop=True)
                        nc.vector.bn_stats(stats[:, kb, :],
                                           up[:, h:h + hs_])
                        kb += 1
                    # copy-back alternates engines to balance load
                    if ci % 3 < 2:
                        nc.scalar.activation(u1[:, off:off + S], up[:, :S],
                                             func=FT.Identity)
                    else:
                        nc.vector.tensor_copy(u1[:, off:off + S], up[:, :S])

                a2, c2 = bn_coeffs(g2_s, be2_s, kb, corr=(v2, v2q))

        # ---- pass C: out = relu(a2*u2+c2) --------------------------------
        with tc.tile_pool(name="op", bufs=6) as op:
            for ci, (off, S) in enumerate(_chunks(ec, 2048)):
                ot = op.tile([P, 2048], bf16, tag="ot")
                if ci % 3 != 2:
                    nc.scalar.activation(ot[:, :S], u1[:, off:off + S],
                                         func=FT.Relu, scale=a2[:],
                                         bias=c2[:])
                else:
                    nc.vector.tensor_scalar(
                        ot[:, :S], u1[:, off:off + S], scalar1=a2[:],
                        scalar2=c2[:], op0=mybir.AluOpType.mult,
                        op1=mybir.AluOpType.add)
                    nc.vector.tensor_scalar_max(ot[:, :S], ot[:, :S], 0.0)
                nc.sync.dma_start(out=outT[:, off:off + S], in_=ot[:, :S])

    # Spread SWDGE gathers over the 4 queues (Q7 cpu pairs) so descriptor
    # generation runs 4-wide.  Assign queue = k % NQ in SCHEDULED order: the
    # tile sem pass assigns completion-sem lanes round-robin (k % 8) over
    # Pool-engine DMA instructions in the same order, so every sem lane sees
    # exactly one queue and completions stay in-order per lane.
    k = 0
    for insts in tc.ordered_instructions_by_block.values():
        for inst in insts:
            if (isinstance(inst, bass_isa.AnyDMAInstruction)
                    and inst.engine == mybir.EngineType.Pool):
                assert isinstance(inst, mybir.InstDMAGatherAnt), type(inst)
                inst.queue_num = k % NQ
                k += 1

    nc.compile()
    return nc


def _wrap16(a):
    """linear [L] -> [16, L/16] wrapped, tiled to [128, L/16]."""
    w = np.ascontiguousarray(a.reshape(-1, 16).T)
    return np.tile(w, (8, 1))


def _perm_cols(xpad):
    """[rows, 128] node-major -> xT columns with the phase-0 permutation:
    within each 1024-node chunk, column s*128+p holds node 8p+s."""
    rows = xpad.shape[0]
    a = xpad.reshape(rows // PH0W, P, PH0W // P, P)   # [j, p, s, f]
    a = a.transpose(0, 2, 1, 3).reshape(rows, P)      # [j, s, p, f]
    return a.T                                        # [128, rows]


def host_prep(x, edge_index, edge_attr, n_cores):
    """Shard edges, bucket-sort, pad; returns per-core arrays + caps."""
    n = x.shape[0]
    e = edge_attr.shape[0]
    ec0 = e // n_cores
    nhi, hi_rows, lo_rows, npad = table_layout(n)

    src_all = edge_index[0].astype(np.int64)
    dst_all = edge_index[1].astype(np.int64)

    per_core = []
    counts = np.zeros((n_cores, 4), np.int64)
    for c in range(n_cores):
        sl = slice(c * ec0, (c + 1) * ec0)
        s, d = src_all[sl], dst_all[sl]
        key = (s >= SPLIT) * 2 + (d >= SPLIT)
        order = np.argsort(key, kind="stable")
        counts[c] = np.bincount(key, minlength=4)
        per_core.append((s, d, key, order))

    caps = tuple(int(max(128, ((counts[:, b].max() + 127) // 128) * 128))
                 for b in range(4))
    ec = sum(caps)
    offs = {}
    _acc = 0
    for b in BUCKET_ORDER:
        offs[b] = _acc
        _acc += caps[b]

    zero_lo = SPLIT          # local zero-row idx in the lo region
    zero_hi = nhi            # local zero-row idx in the hi region

    cores = []
    for c in range(n_cores):
        s, d, key, order = per_core[c]
        cnt = counts[c]
        pos_sorted = np.empty(ec0, np.int64)
        start = 0
        sidx_p = np.empty(ec, np.int64)
        didx_p = np.empty(ec, np.int64)
        ea_cols = np.full(ec, -1, np.int64)  # source edge for each padded col
        for b in range(4):
            idx_b = order[start:start + cnt[b]]
            pos = offs[b] + np.arange(cnt[b])
            pos_sorted[start:start + cnt[b]] = pos
            sb = s[idx_b]
            db = d[idx_b]
            src_hi, dst_hi = b >= 2, b % 2 == 1
            sl_loc = sb - SPLIT if src_hi else sb
            dl_loc = db - SPLIT if dst_hi else db
            sidx_p[pos] = sl_loc
            didx_p[pos] = dl_loc
            ea_cols[pos] = idx_b
            padr = np.arange(offs[b] + cnt[b], offs[b] + caps[b])
            sidx_p[padr] = zero_hi if src_hi else zero_lo
            didx_p[padr] = zero_hi if dst_hi else zero_lo
            start += cnt[b]
        inv = np.empty(ec0, np.int64)
        inv[order] = pos_sorted  # padded position of original local edge
        cores.append((sidx_p.astype(np.int16), didx_p.astype(np.int16),
                      ea_cols, inv))
    return caps, ec, cores, npad


def make_in_maps(x, edge_index, edge_attr, W_lin, b_lin, W1, g1, be1, W2,
                 g2, be2, n_cores):
    n = x.shape[0]
    nhi, hi_rows, lo_rows, npad = table_layout(n)
    caps, ec, cores, _ = host_prep(x, edge_index, edge_attr, n_cores)

    # xT columns: [0, lo_rows) lo nodes, [lo_rows, npad) hi nodes, both with
    # the per-1024-chunk phase-0 permutation.
    xbf = x.astype(BF16)
    xlo = np.zeros((lo_rows, P), dtype=BF16)
    xlo[:SPLIT] = xbf[:SPLIT]
    xhi = np.zeros((hi_rows, P), dtype=BF16)
    xhi[:nhi] = xbf[SPLIT:n]
    xT = np.concatenate([_perm_cols(xlo), _perm_cols(xhi)], axis=1)
    xT = np.ascontiguousarray(xT)

    # host-side weight prep (f32)
    W_lin = np.asarray(W_lin, np.float32)
    b_lin = np.asarray(b_lin, np.float32)
    W1 = np.asarray(W1, np.float32)
    W2 = np.asarray(W2, np.float32)
    W1a = W1[:, :P]
    W1b = W1[:, P:]
    Wc = W1a @ W_lin                     # [nout, nin]
    bc_h = (W1a @ b_lin).astype(np.float32).reshape(P, 1)
    wct_h = np.ascontiguousarray(Wc.T).astype(BF16)
    w1bt_h = np.ascontiguousarray(W1b.T).astype(BF16)
    w2t_h = np.ascontiguousarray(W2.T).astype(BF16)
    ident_h = np.eye(P, dtype=BF16)

    f32c = np.ascontiguousarray
    g1_h = f32c(np.asarray(g1, np.float32).reshape(P, 1))
    be1_h = f32c(np.asarray(be1, np.float32).reshape(P, 1))
    g2_h = f32c(np.asarray(g2, np.float32).reshape(P, 1))
    be2_h = f32c(np.asarray(be2, np.float32).reshape(P, 1))

    groups, _ = edge_layout(caps)
    eabf = edge_attr.astype(BF16)

    in_maps = []
    invs = []
    for c in range(n_cores):
        sidx_p, didx_p, ea_cols, inv = cores[c]
        ec0 = inv.shape[0]
        eaT = np.zeros((P, ec), dtype=BF16)
        real = ea_cols >= 0
        eaT[:, real] = eabf[c * ec0 + ea_cols[real]].T
        sw = np.zeros((P, ec // 16), np.int16)
        dw = np.zeros((P, ec // 16), np.int16)
        for off, L, _, _ in groups:
            sw[:, off // 16:(off + L) // 16] = _wrap16(sidx_p[off:off + L])
            dw[:, off // 16:(off + L) // 16] = _wrap16(didx_p[off:off + L])
        in_maps.append({
            "eaT": eaT, "xT": xT, "sidx": sw, "didx": dw,
            "wct": wct_h, "w1bt": w1bt_h, "w2t": w2t_h, "ident": ident_h,
            "bc": bc_h, "g1": g1_h, "be1": be1_h, "g2": g2_h, "be2": be2_h,
        })
        invs.append(inv)
    return caps, ec, in_maps, invs


_GRAPH_CACHE = {}


def get_graph(n_cores, caps, n_nodes, e_total):
    key = (n_cores, caps, n_nodes, e_total)
    if key not in _GRAPH_CACHE:
        _GRAPH_CACHE[key] = build_graph(n_cores, caps, n_nodes, e_total)
    return _GRAPH_CACHE[key]


def kernel(x, edge_index, edge_attr, W_lin, b_lin, W1, b1, g1, be1, W2, b2,
           g2, be2):
    """Full-input entry point: shard, run on 8 NeuronCores, gather."""
    x = np.asarray(x)
    edge_index = np.asarray(edge_index)
    edge_attr = np.asarray(edge_attr)
    e = edge_attr.shape[0]
    n = x.shape[0]
    ec0 = e // N_CORES

    caps, ec, in_maps, invs = make_in_maps(
        x, edge_index, edge_attr, np.asarray(W_lin), np.asarray(b_lin),
        np.asarray(W1), np.asarray(g1), np.asarray(be1), np.asarray(W2),
        np.asarray(g2), np.asarray(be2), N_CORES)
    nc = get_graph(N_CORES, caps, n, e)
    res = run_bass_kernel_spmd(nc, in_maps, core_ids=list(range(N_CORES)))
    out = np.empty((e, NIN), dtype=np.float32)
    for c in range(N_CORES):
        oT = np.asarray(res.results[c]["outT"], dtype=np.float32)
        out[c * ec0:(c + 1) * ec0] = oT.T[invs[c]]
    return out
